# revision 3
# baseline (speedup 1.0000x reference)
"""Trainium2 Bass kernel for nn_CapsuleLayer (dynamic-routing capsule layer).

Reference computation:
    priors = einsum('bni,cnio->cbno', x, W)            # [32c, 64b, 1152n, 32o]
    3 iterations of routing over n (capsules):
        probs = softmax(logits, axis=n)
        s = sum_n(probs * priors);  outputs = squash(s)
        logits += sum_o(priors * outputs)
    return outputs  # [32, 64, 1, 1, 32]

Sharding: 8 cores x 4 classes each (routing is independent per (class,batch));
class sharding minimizes W traffic (each core reads only its 18.9MB W slice).

Structure per core: 2x (priors phase for a class-pair -> routing both classes).
L (the pair's priors, fp32 [128=(par,b), 2c, 32o, 576nr], 144KB/partition)
fills most of SBUF, which forces the phases to serialize.

Priors phase (~120us/pair):
  - fp16 3-term split-precision matmuls: one K=96 self-loading matmul per
    capsule computes x_hi*W_hi + x_lo*W_hi + x_hi*W_lo via K-concatenation
    (rows [x_hi; x_lo; x_hi] x [W_hi; W_hi; W_lo]); residual ~2^-22, end
    to end rel err 5.6e-5.  fp16 matmuls run 1 cycle/row vs fp32's 4 (two
    half-speed HW matmuls), taking TensorE from ~1.45ms busy to ~104us.
  - K padded to 128 with zero rows (walrus rejects K=96 Ldweights vs the
    rounded-up tile size); pad rows live in persistent ping-pong chunk
    tiles (depth 3), memset once.
  - The rust scheduler splits 16-bit matmuls into standalone prefetch
    Ldweights + Matmult(ldweights=False); this walrus rejects standalone
    Ldweights, so _merge_ldweights() folds them back to self-loading form.
  - One DMA per 32-capsule chunk per operand, per-chunk-contiguous host
    layout (4KB/partition runs); x on the SP HWDGE queue, W on the
    Activation HWDGE queue (small strided DMAs left the DGE overhead-bound
    at ~140GB/s effective; 786KB contiguous DMAs reach ~435GB/s bursts).
  - Drains psum->L alternate DVE/ACT.

Routing (~270us/pair), all fp32 (fp16 L / probs / logits all measured at
1.4e-2..4e-2 vs the 2e-2 gate -- routing amplifies error exponentially via
softmax logits up to +-80):
  - The two classes of a pair are issued interleaved, stage by stage
    (generators alternating at engine-batch boundaries) so one class's
    slice batches fill the other's serial fold->squash->delta gaps.
  - it0 s-pass is a plain mean: class 0 runs 16 o-slices on DVE + 16 on
    ACT, class 1 all 32 on ACT (activation Copy w/ scale + accum_out), so
    both classes start immediately on idle engines.
  - it1/it2 s-passes: DVE fused STT (out discarded, accum_out = sum).
  - delta passes: DVE STT chain for o < DVE_O=15, GPSIMD mult-broadcast +
    add chain for the rest (GPSIMD has no working fused STT and cannot
    read PSUM), one final DVE add folds logits += delta for free.
  - Parity-fold + broadcast via one matmul with sel2[128,128]=tile(I_64);
    exp on ACT with accum_out gives the softmax normalizer for free;
    squash factor sqrt(sq)/(1+sq) via Ln/Exp + vector reciprocal.

Measured on 8xtrn2: 869us (baseline fp32 kernel: 1034us).
"""

from contextlib import ExitStack

import numpy as np

import concourse.bass as bass
import concourse.mybir as mybir
import concourse.tile as tile
from concourse import bass_utils

AF = mybir.ActivationFunctionType
ALU = mybir.AluOpType
FP32 = mybir.dt.float32
FP16 = mybir.dt.float16

C, B, N, I, O = 32, 64, 1152, 32, 32
N_CORES = 8
CLASSES_PER_CORE = C // N_CORES          # 4
PAIRS = CLASSES_PER_CORE // 2            # 2 class-pairs per core
NR = N // 2                              # 576 n-pairs (n = 2*nr + parity)
T = N // 4                               # 288 4-n host blocks
K3 = 3 * I                               # 96 real rows = concat of 3 fp16 terms
K4 = 128                                 # padded K (rows 96-127 zero): walrus
                                         # rejects Ldweights with K=96 vs the
                                         # rounded-up tile_size 128
INV_N = 1.0 / N

T_CHUNK = 8                              # t-blocks per DMA chunk (32 n)
N_CHUNKS = T // T_CHUNK                  # 36 chunks = 36 drains per pair
SLOTS = 16                               # nr slots per psum drain (32 n)
# delta o-slice split between DVE (fused STT) and GPSIMD (mult+add):
# DVE STT ~1.07us/slice vs GPSIMD 2-op ~3.3us/slice; 12 slices on GPSIMD
# balances engine budgets without making the GP chain the critical path.
DVE_O = 15                               # o-slices on DVE; rest on GPSIMD


def _legalize_syncs(nc):
    """This container's walrus codegen allows at most ONE sync-wait command
    per instruction, and a `sem-eq-imm` wait encodes as two commands.  Strip
    vacuous eq-0 drain waits and hoist all-but-one ge-waits onto injected
    same-engine NoOps."""
    nid = 0
    for f in nc.m.functions:
        for blk in f.blocks:
            newlist = []
            changed = False
            for ins in blk.instructions:
                si = ins.sync_info
                if si is None or not si.on_wait:
                    newlist.append(ins)
                    continue
                waits = list(si.on_wait)
                if isinstance(ins, mybir.InstDrain):
                    kept = [w for w in waits
                            if not (w.wait_mode == "sem-eq-imm"
                                    and w.wait_value == 0)]
                    if len(kept) != len(waits):
                        changed = True
                    waits = kept
                while len(waits) > 1:
                    w = waits.pop(0)
                    ev = mybir.InstEventSemaphore(
                        name=f"syncfix_{nid}", ins=[], outs=[])
                    nid += 1
                    ev.engine = ins.engine
                    ev.sync_info = mybir.SyncInfo(on_wait=[w], on_update=[])
                    newlist.append(ev)
                    changed = True
                ins.sync_info = mybir.SyncInfo(on_wait=waits,
                                               on_update=list(si.on_update))
                newlist.append(ins)
            if changed:
                blk.instructions = newlist
    return nc


def _merge_ldweights(nc):
    """The rust scheduler splits 16-bit matmuls into a standalone prefetch
    InstLdweights + InstMatmult(ldweights=False), but this walrus rejects
    standalone Ldweights ("not compatible with LDW optimization").  Merge
    back to the self-loading form (ldweights=None, like fp32 takes): drop
    the Ldweights, keep its sync waits on an injected same-engine NoOp so
    the matmul still waits for its operand DMAs."""
    nid = 0
    for f in nc.m.functions:
        for blk in f.blocks:
            newl = []
            changed = False
            for ins in blk.instructions:
                if isinstance(ins, mybir.InstLdweights):
                    si = ins.sync_info
                    if si is not None and (si.on_wait or si.on_update):
                        ev = mybir.InstEventSemaphore(
                            name=f"ldwfix_{nid}", ins=[], outs=[])
                        nid += 1
                        ev.engine = ins.engine
                        ev.sync_info = si
                        newl.append(ev)
                    changed = True
                    continue
                if isinstance(ins, mybir.InstMatmult) and ins.ldweights is False:
                    ins.ldweights = None
                    changed = True
                newl.append(ins)
            if changed:
                blk.instructions = newl
    return nc


def _build_program(nc: bass.Bass, legalize: bool = True):
    # xT[t, r=96, g, b]: rows 0-31 x_hi^T, 32-63 x_lo^T, 64-95 x_hi^T
    # Wr[pair, t, r=96, g, (cc,o)]: rows 0-31 W_hi, 32-63 W_hi, 64-95 W_lo
    # => psum = x_hi*W_hi + x_lo*W_hi + x_hi*W_lo  (one matmul per n)
    # per-chunk-contiguous layouts: one DMA per chunk per operand, 4KB
    # contiguous per partition (the previous 16n strided DMAs left the DGE
    # overhead-bound at ~140 GB/s effective; big contiguous DMAs hit ~435)
    xT = nc.dram_tensor("xT", [N_CHUNKS, K3, T_CHUNK, 4, B], FP16,
                        kind="ExternalInput").ap()
    Wr = nc.dram_tensor("Wr", [PAIRS, N_CHUNKS, K3, T_CHUNK, 4, 64], FP16,
                        kind="ExternalInput").ap()
    sel2 = nc.dram_tensor("sel2", [128, 128], FP32, kind="ExternalInput").ap()
    out4 = nc.dram_tensor("out4", [CLASSES_PER_CORE, B, O], FP32,
                          kind="ExternalOutput").ap()

    with tile.TileContext(nc) as tc, ExitStack() as ctx:  # noqa: SIM117
        pools = {
            "consts": ctx.enter_context(tc.tile_pool(name="consts", bufs=1)),
            "lpool": ctx.enter_context(tc.tile_pool(name="lpool", bufs=1)),
            "pr_psum": ctx.enter_context(
                tc.tile_pool(name="pr_psum", bufs=3, space="PSUM")),
            "fold_psum": ctx.enter_context(
                tc.tile_pool(name="fold_psum", bufs=1, space="PSUM")),
            "scratch": ctx.enter_context(tc.tile_pool(name="scratch", bufs=1)),
            "small": ctx.enter_context(tc.tile_pool(name="small", bufs=2)),
        }
        nc_ = tc.nc
        sel_t = pools["consts"].tile([128, 128], FP32)
        nc_.sync.dma_start(out=sel_t[:], in_=sel2)

        # persistent ping-pong chunk tiles so the K-pad rows (96-127) can be
        # zeroed ONCE and stay zero (pool-fresh tiles would need re-zeroing)
        xw_tiles = []
        for i in range(3):
            xt = pools["consts"].tile([K4, T_CHUNK, 4, B], FP16,
                                      tag=f"xch{i}", name=f"xch{i}")
            wt = pools["consts"].tile([K4, T_CHUNK, 4, 64], FP16,
                                      tag=f"wch{i}", name=f"wch{i}")
            nc_.vector.memset(xt[K3:K4], 0.0)
            nc_.vector.memset(wt[K3:K4], 0.0)
            xw_tiles.append((xt, wt))
        assert T_CHUNK * 4 == 2 * SLOTS

        # priors for one class-pair: [(par,b)=128, (cc,o,nr)] fp32, 144KB/part
        L = pools["lpool"].tile([128, 2, O, NR], FP32, tag="L")

        for pair in range(PAIRS):
            _priors_phase(tc, xT, Wr, pair, L, pools, xw_tiles)
            # interleave the two classes stage-by-stage: while one class's
            # serial chain (fold/squash) runs on ACT/PE, the other's slice
            # batches keep DVE/GPSIMD fed.  cc=0 takes its it0 s-pass on
            # DVE, cc=1 on ACT, so both start immediately in parallel.
            gens = [_route_class(tc, L, sel_t, cc, pair, out4, pools)
                    for cc in range(2)]
            alive = list(gens)
            while alive:
                for g in list(alive):
                    try:
                        next(g)
                    except StopIteration:
                        alive.remove(g)
    _merge_ldweights(nc)
    if legalize:
        _legalize_syncs(nc)
    return nc


def _priors_phase(tc, xT, Wr, pair, L, pools, xw_tiles):
    nc = tc.nc
    # one chunk = one x DMA (sync queue) + one W DMA (scalar queue) = 32 n
    # = one psum drain
    for chunk in range(N_CHUNKS):
        psum_t = pools["pr_psum"].tile([128, SLOTS, 64], FP32, tag="prpsum")
        x_tile, w_tile = xw_tiles[chunk % 3]
        nc.sync.dma_start(out=x_tile[0:K3], in_=xT[chunk])
        nc.scalar.dma_start(out=w_tile[0:K3], in_=Wr[pair, chunk])
        for tt in range(T_CHUNK):
            for g in range(4):
                slot = (tt * 4 + g) // 2
                par = g & 1                # n parity = g parity (n = 4t+g)
                nc.tensor.matmul(
                    psum_t[par * 64:(par + 1) * 64, slot, :],
                    x_tile[:, tt, g, :],
                    w_tile[:, tt, g, :],
                    start=True, stop=True,
                    tile_position=(0, par * 64))
        lview = L[:, :, :, chunk * SLOTS:(chunk + 1) * SLOTS]
        lview = lview.rearrange("p c o s -> p s c o")
        src = psum_t[:].rearrange("p s (c o) -> p s c o", c=2)
        if chunk % 2 == 0:
            nc.vector.tensor_copy(lview, src)
        else:
            nc.scalar.copy(lview, src)


def _route_class(tc, L, sel_t, cc, pair, out4, pools):
    nc = tc.nc
    scratch, small, fold_psum = pools["scratch"], pools["small"], pools["fold_psum"]
    Lc = L[:, cc]                          # [128, 32, 576]

    junk = scratch.tile([128, NR], FP32, tag=f"junk{cc}", name=f"junk{cc}")
    junka = scratch.tile([128, NR], FP32, tag=f"junka{cc}", name=f"junka{cc}")
    e_t = scratch.tile([128, NR], FP32, tag=f"e{cc}", name=f"e{cc}")
    lG = scratch.tile([128, NR], FP32, tag=f"lG{cc}", name=f"lG{cc}")
    gtmp = scratch.tile([128, NR], FP32, tag=f"gtmp{cc}", name=f"gtmp{cc}")
    dbufs = [scratch.tile([128, NR], FP32, tag=f"ld{cc}_{i}", name=f"ld{cc}_{i}")
             for i in range(2)]
    sp = small.tile([128, O], FP32, tag=f"sp{cc}")
    zp = small.tile([128, 1], FP32, tag=f"zp{cc}")

    def fold(src_ap, ncols, tag):
        """[128=(par,b), ncols] -> [128, ncols] (parity-summed, b-replicated
        across both partition halves) via sel2 matmul; lands in SBUF."""
        ps = fold_psum.tile([128, 64], FP32, tag=f"foldps{cc}")
        nc.tensor.matmul(ps[:, :ncols], sel_t[:], src_ap,
                         start=True, stop=True)
        dst = small.tile([128, ncols], FP32, tag=tag)
        nc.scalar.copy(dst[:], ps[:, :ncols])
        return dst

    def s_accumulate_uniform():
        """it0: sp[:, o] = (1/N) sum_nr Lc[:, o, :].  Split so neither
        engine idles at phase start: class 0 = half DVE half ACT (finishes
        fast, unblocks its fold/delta), class 1 = all ACT."""
        for o in range(O):
            if cc == 0 and o < 16:
                nc.vector.tensor_scalar(
                    out=junk[:], in0=Lc[:, o, :], scalar1=INV_N, scalar2=None,
                    op0=ALU.mult, op1=ALU.add, accum_out=sp[:, o:o + 1])
            else:
                nc.scalar.activation(junka[:], Lc[:, o, :], AF.Copy,
                                     scale=INV_N, accum_out=sp[:, o:o + 1])

    def s_accumulate(weight):
        """sp[:, o] = sum_nr Lc[:, o, :] * weight[:, nr] on DVE."""
        for o in range(O):
            nc.vector.scalar_tensor_tensor(
                out=junk[:], in0=Lc[:, o, :], scalar=0.0, in1=weight[:],
                op0=ALU.bypass, op1=ALU.mult, accum_out=sp[:, o:o + 1])

    def delta_accumulate(outs, prev):
        """Return tile holding (prev-logits or 0) + sum_o Lc[:,o,:]*outs[:,o].
        DVE chains o<DVE_O (seeded with prev), GPSIMD chains the rest into lG."""
        cur = prev
        for o in range(DVE_O):
            dst = dbufs[0] if cur is not dbufs[0] else dbufs[1]
            if cur is None:
                nc.vector.scalar_tensor_tensor(
                    out=dst[:], in0=Lc[:, o, :], scalar=outs[:, o:o + 1],
                    in1=Lc[:, o, :], op0=ALU.mult, op1=ALU.bypass)
            else:
                nc.vector.scalar_tensor_tensor(
                    out=dst[:], in0=Lc[:, o, :], scalar=outs[:, o:o + 1],
                    in1=cur[:], op0=ALU.mult, op1=ALU.add)
            cur = dst
        first = True
        for o in range(DVE_O, O):
            # gpsimd lacks a working fused scalar*tensor+tensor: do
            # mult (per-partition scalar broadcast) then accumulate-add.
            if first:
                nc.gpsimd.tensor_mul(lG[:], Lc[:, o, :],
                                     outs[:, o:o + 1].to_broadcast((128, NR)))
                first = False
            else:
                nc.gpsimd.tensor_mul(gtmp[:], Lc[:, o, :],
                                     outs[:, o:o + 1].to_broadcast((128, NR)))
                nc.gpsimd.tensor_add(lG[:], lG[:], gtmp[:])
        nc.vector.tensor_add(cur[:], cur[:], lG[:])
        return cur

    def squash(s_sb):
        """outputs = s * sqrt(sq)/(1+sq), all [128, O] per-partition."""
        sq = small.tile([128, 1], FP32, tag=f"sq{cc}")
        nc.vector.scalar_tensor_tensor(
            out=junk[:, :O], in0=s_sb[:], scalar=0.0, in1=s_sb[:],
            op0=ALU.bypass, op1=ALU.mult, accum_out=sq[:])
        lnq = small.tile([128, 1], FP32, tag=f"lnq{cc}")
        nc.scalar.activation(lnq[:], sq[:], AF.Ln)
        r = small.tile([128, 1], FP32, tag=f"r{cc}")
        nc.scalar.activation(r[:], lnq[:], AF.Exp, scale=0.5)  # sqrt(sq)
        q1 = small.tile([128, 1], FP32, tag=f"q1{cc}")
        nc.vector.tensor_scalar_add(q1[:], sq[:], 1.0)
        iq = small.tile([128, 1], FP32, tag=f"iq{cc}")
        nc.vector.reciprocal(iq[:], q1[:])
        f = small.tile([128, 1], FP32, tag=f"f{cc}")
        nc.vector.tensor_mul(f[:], r[:], iq[:])
        outs = small.tile([128, O], FP32, tag=f"outs{cc}")
        nc.vector.tensor_scalar(out=outs[:], in0=s_sb[:], scalar1=f[:],
                                scalar2=None, op0=ALU.mult)
        return outs

    # iteration 0: s0 = mean_n p (DVE for cc=0, ACT for cc=1)
    s_accumulate_uniform()
    yield
    s_sb = fold(sp[:], O, f"s_sb{cc}")
    outs = squash(s_sb)
    yield
    logits = delta_accumulate(outs, None)
    yield

    for it in (1, 2):
        nc.scalar.activation(e_t[:], logits[:], AF.Exp, accum_out=zp[:])
        s_accumulate(e_t)
        yield
        z_sb = fold(zp[:], 1, f"z_sb{cc}")
        iz = small.tile([128, 1], FP32, tag=f"iz{cc}")
        nc.vector.reciprocal(iz[:], z_sb[:])
        sraw = fold(sp[:], O, f"s_sb{cc}")
        s_sb = small.tile([128, O], FP32, tag=f"s_n{cc}")
        nc.vector.tensor_scalar(out=s_sb[:], in0=sraw[:], scalar1=iz[:],
                                scalar2=None, op0=ALU.mult)
        outs = squash(s_sb)
        yield
        if it == 1:
            logits = delta_accumulate(outs, logits)
            yield
        else:
            nc.sync.dma_start(out=out4[pair * 2 + cc], in_=outs[:B, :])


# ---------------------------------------------------------------------------
# host-side entry point
# ---------------------------------------------------------------------------

_COMPILED = {}


def _split16(a):
    hi = a.astype(np.float16)
    lo = (a - hi.astype(np.float32)).astype(np.float16)
    return hi, lo


def _prep_host_inputs(x, route_weights):
    x = np.ascontiguousarray(x, dtype=np.float32)
    W = np.ascontiguousarray(route_weights, dtype=np.float32)
    # xT3[t, r, g, b]: r rows = [x_hi^T; x_lo^T; x_hi^T]
    xh, xl = _split16(x)                                  # [B, N, I] fp16

    def xrows(a):                                         # -> [T, I, 4, B]
        return a.reshape(B, T, 4, I).transpose(1, 3, 2, 0)
    xT3 = np.concatenate([xrows(xh), xrows(xl), xrows(xh)], axis=1)
    # -> per-chunk contiguous [N_CHUNKS, 96, T_CHUNK, 4, B]
    xT3 = xT3.reshape(N_CHUNKS, T_CHUNK, K3, 4, B).transpose(0, 2, 1, 3, 4)
    xT3 = np.ascontiguousarray(xT3)

    sel2 = np.tile(np.eye(B, dtype=np.float32), (2, 2)).astype(np.float32)
    sel2 = np.ascontiguousarray(sel2)

    in_maps = []
    for k in range(N_CORES):
        Wk = W[k * CLASSES_PER_CORE:(k + 1) * CLASSES_PER_CORE]
        Wh, Wl = _split16(Wk)                             # [4, N, I, O] fp16

        def wrows(a):                                     # -> [P, T, I, 4, 64]
            r = a.reshape(PAIRS, 2, T, 4, I, O).transpose(0, 2, 4, 3, 1, 5)
            return r.reshape(PAIRS, T, I, 4, 64)
        Wr3 = np.concatenate([wrows(Wh), wrows(Wh), wrows(Wl)], axis=2)
        # -> per-chunk contiguous [P, N_CHUNKS, 96, T_CHUNK, 4, 64]
        Wr3 = Wr3.reshape(PAIRS, N_CHUNKS, T_CHUNK, K3, 4, 64)
        Wr3 = Wr3.transpose(0, 1, 3, 2, 4, 5)
        Wr3 = np.ascontiguousarray(Wr3)
        in_maps.append({"xT": xT3, "Wr": Wr3, "sel2": sel2})
    return in_maps


def _get_compiled():
    if "nc" not in _COMPILED:
        nc = bass.Bass("TRN2", target_bir_lowering=False, debug=False,
                       enable_asserts=False, num_devices=N_CORES)
        _build_program(nc)
        _COMPILED["nc"] = nc
    return _COMPILED["nc"]


def kernel(x, route_weights, **run_kwargs):
    in_maps = _prep_host_inputs(x, route_weights)
    nc = _get_compiled()
    res = bass_utils.run_bass_kernel_spmd(
        nc, in_maps, core_ids=list(range(N_CORES)), **run_kwargs)
    full = np.concatenate([r["out4"] for r in res.results], axis=0)
    out = full[:, :, None, None, :].astype(np.float32)
    if run_kwargs:
        kernel.last_results = res
    return out


if __name__ == "__main__":
    rng = np.random.default_rng(0)
    xs = rng.standard_normal((B, N, I), dtype=np.float32)
    ws = rng.standard_normal((C, N, I, O), dtype=np.float32)
    print(kernel(xs, ws).shape)
